# revision 1
# baseline (speedup 1.0000x reference)
"""TT-dense layer (BayesKerasDense): y = relu(x @ M + b), M given as a
4-core tensor-train. Strategy: the TT ranks (16) are large relative to the
mode sizes (8), so the TT sweep costs as many FLOPs as the dense matmul but
with 16x-larger intermediates and a full re-layout between stages. We
therefore materialize the dense M = TT(core0..core3) once on the host
(0.34 GMAC, trivial) and run a data-parallel dense matmul on 8 NeuronCores:
each core computes y_local[512, 4096] = relu(xT_local.T @ M + b) in bf16
with fp32 PSUM accumulation. The bias is folded into the accumulation as a
K=1 matmul (ones.T @ b); relu is fused into the PSUM->SBUF evacuation on
the scalar/vector engines.
"""

import sys

import numpy as np
import ml_dtypes

try:
    import concourse.bacc as bacc
except ImportError:  # fallback for environments without the site hook
    sys.path.insert(0, "/opt/trn_rl_repo")
    import concourse.bacc as bacc
import concourse.mybir as mybir
import concourse.tile as tile
from concourse.bass_utils import run_bass_kernel_spmd

N_CORES = 8
B = 4096          # global batch
BL = B // N_CORES # per-core batch (512)
D = 4096          # n_in == n_out
BF16 = mybir.dt.bfloat16
F32 = mybir.dt.float32

N_TILES = D // 512    # 8 column tiles of 512
K_TILES = D // 128    # 32 contraction chunks of 128
M_TILES = BL // 128   # 4 batch tiles of 128


def _build_module(
    mpool_bufs: int = 10,
    xt_mode: str = "swdge_each",
    split_last_n: bool = False,
    bias_mode: str = "evac",
    prefetch_mt: int = 0,
    mt_pair: bool = False,
    b0_engine: str = "scalar",
    last_m_outer: bool = False,
    warmup_mms: int = 0,
    first_tile_bias_matmul: bool = True,
    layout: str = "batch_part",
    fuse_first_pair: bool = True,
):
    if layout == "feat_part":
        return _build_module_featpart(mpool_bufs)
    nc = bacc.Bacc("TRN2", target_bir_lowering=False, debug=False, num_devices=N_CORES)
    xt_d = nc.dram_tensor("xt", [D, BL], BF16, kind="ExternalInput")
    mw_d = nc.dram_tensor("mw", [D, D], BF16, kind="ExternalInput")
    b_shape = [128, D] if bias_mode == "evac" else [D]
    b_d = nc.dram_tensor("bv", b_shape, BF16, kind="ExternalInput")
    y_d = nc.dram_tensor("y", [BL, D], F32, kind="ExternalOutput")

    with tile.TileContext(nc) as tc:
        with (
            tc.tile_pool(name="const", bufs=1) as cpool,
            tc.tile_pool(name="mpool", bufs=mpool_bufs) as mpool,
            tc.tile_pool(name="mlast", bufs=K_TILES + 1) as mlast_pool,
            tc.tile_pool(name="ypool", bufs=4) as ypool,
            tc.tile_pool(name="pspool", bufs=8, space="PSUM") as pspool,
        ):
            # x^T resident in SBUF: [128 partitions, K_TILES, BL] bf16.
            # Loads are interleaved with the n=0 M-tile stream so the first
            # matmuls aren't head-of-line blocked behind the whole 4MB.
            xt_sb = cpool.tile([128, K_TILES, BL], BF16)
            if bias_mode == "evac":
                # single-row bias for the first/last col-tiles' bias matmuls.
                # Only the first pair's slice is on the startup critical
                # path; the rest loads behind it.
                b0_sb = cpool.tile([1, D], BF16)
                # 2KB at the SWDGE queue head: unblocks the bias matmuls
                # earliest; costs xt[1] almost nothing
                b0_eng = nc.gpsimd if b0_engine == "scalar" else nc.sync
                b0_eng.dma_start(out=b0_sb[:, 0:1024], in_=b_d[0:1, 0:1024])
                # full replicated bias for the evacuation adds; DMA emission
                # deferred until after the n=0 tile stream so it doesn't
                # head-of-line block the first matmuls' inputs.
                b_sb = cpool.tile([128, D], BF16)
            else:
                b_sb = cpool.tile([1, D], BF16)
                nc.sync.dma_start(out=b_sb[:, :], in_=b_d[None, :])
                b0_sb = b_sb
            ones_sb = cpool.tile([1, 128], BF16)
            nc.vector.memset(ones_sb[:], 1.0)

            # discarded matmuls with no DMA dependencies: keep the PE busy
            # from t~0 while the first x/M tiles arrive, tripping the clock
            # ramp earlier
            for w in range(warmup_mms):
                wps = pspool.tile([128, 512], F32, name=f"wps_{w}", tag="ps")
                nc.tensor.matmul(
                    wps[:], ones_sb[:, 0:128], ones_sb[:, :],
                    start=True, stop=True,
                )

            def load_xt(k):
                if xt_mode == "swdge_each":
                    # k=0 on HWDGE (fast first-byte) so the first matmuls
                    # start ASAP; bulk on SWDGE in parallel with the M
                    # stream; tail back on HWDGE which has slack by then
                    # (SWDGE alone delivers ~1.04us/tile > the 0.85us/tile
                    # consumption rate and would starve the last k-steps)
                    eng = nc.sync if (k == 0 or k >= 28) else nc.gpsimd
                    eng.dma_start(
                        out=xt_sb[:, k, :], in_=xt_d[k * 128 : (k + 1) * 128, :]
                    )
                elif xt_mode == "split":
                    eng = nc.sync if k % 2 == 0 else nc.gpsimd
                    eng.dma_start(
                        out=xt_sb[:, k, :], in_=xt_d[k * 128 : (k + 1) * 128, :]
                    )
                elif xt_mode == "chunk_hybrid":
                    # head fine-grained for fast start, middle in 4-chunk
                    # SWDGE loads (amortized launch overhead), tail on HWDGE
                    src = xt_d.rearrange("(c p) b -> p c b", p=128)
                    if k == 0 or k >= 28:
                        nc.sync.dma_start(
                            out=xt_sb[:, k, :],
                            in_=xt_d[k * 128 : (k + 1) * 128, :],
                        )
                    elif k in (1, 2, 3):
                        nc.gpsimd.dma_start(
                            out=xt_sb[:, k, :],
                            in_=xt_d[k * 128 : (k + 1) * 128, :],
                        )
                    elif k % 4 == 0:
                        nc.gpsimd.dma_start(
                            out=xt_sb[:, k : k + 4, :], in_=src[:, k : k + 4, :]
                        )
                elif xt_mode == "swdge_chunk4":
                    if k % 4 == 0:
                        src = xt_d.rearrange("(c p) b -> p c b", p=128)
                        nc.gpsimd.dma_start(
                            out=xt_sb[:, k : k + 4, :], in_=src[:, k : k + 4, :]
                        )
                else:
                    raise ValueError(xt_mode)

            # (n-tile index, column offset, column width)
            col_tiles = []
            for n in range(N_TILES):
                if split_last_n and n == N_TILES - 1:
                    col_tiles.append((n, n * 512, 384))
                    col_tiles.append((n, n * 512 + 384, 128))
                else:
                    col_tiles.append((n, n * 512, 512))

            def emit_store(ci, m, ns, cw, ps_m, yt):
                if ci == len(col_tiles) - 1:
                    # tail stores: distinct launch queues so the HW DMA
                    # engines drain them in parallel
                    dma_eng = (nc.sync, nc.gpsimd, nc.scalar, nc.sync)[m]
                else:
                    dma_eng = (nc.sync, nc.gpsimd, nc.scalar, nc.gpsimd)[m]
                dma_eng.dma_start(
                    out=y_d[m * 128 : (m + 1) * 128, ns], in_=yt[:, :cw]
                )

            if fuse_first_pair and not split_last_n and bias_mode == "evac":
                # Joint k-loop over the first two col-tiles: 8 matmuls per
                # k-step consume xt at 1.7us/tile (vs the ~1.04us/tile SWDGE
                # delivery), so the x^T preload always stays ahead. Uses all
                # 8 PSUM banks for the duration.
                psA = [
                    pspool.tile([128, 512], F32, name=f"psA_{m}", tag="ps")
                    for m in range(M_TILES)
                ]
                psB = [
                    pspool.tile([128, 512], F32, name=f"psB_{m}", tag="ps")
                    for m in range(M_TILES)
                ]
                nsA, nsB = slice(0, 512), slice(512, 1024)
                if first_tile_bias_matmul:
                    for m in range(M_TILES):
                        nc.tensor.matmul(
                            psA[m][:], ones_sb[:, 0:128], b0_sb[0:1, nsA],
                            start=True, stop=False,
                        )
                        nc.tensor.matmul(
                            psB[m][:], ones_sb[:, 0:128], b0_sb[0:1, nsB],
                            start=True, stop=False,
                        )
                for k in range(K_TILES):
                    load_xt(k)
                    mtA = mpool.tile([128, 512], BF16, name=f"mtA_{k}", tag="mt")
                    nc.sync.dma_start(
                        out=mtA[:], in_=mw_d[k * 128 : (k + 1) * 128, nsA]
                    )
                    mtB = mpool.tile([128, 512], BF16, name=f"mtB_{k}", tag="mt")
                    nc.sync.dma_start(
                        out=mtB[:], in_=mw_d[k * 128 : (k + 1) * 128, nsB]
                    )
                    for m in range(M_TILES):
                        nc.tensor.matmul(
                            psA[m][:],
                            xt_sb[:, k, m * 128 : (m + 1) * 128],
                            mtA[:],
                            start=(not first_tile_bias_matmul and k == 0),
                            stop=(k == K_TILES - 1),
                        )
                        nc.tensor.matmul(
                            psB[m][:],
                            xt_sb[:, k, m * 128 : (m + 1) * 128],
                            mtB[:],
                            start=(not first_tile_bias_matmul and k == 0),
                            stop=(k == K_TILES - 1),
                        )
                # rest of the single-row bias (for the last tile's bias
                # matmuls) + bias slices for the middle tiles: SWDGE FIFO,
                # behind the xt stream
                nc.gpsimd.dma_start(out=b0_sb[:, 1024:], in_=b_d[0:1, 1024:])
                for bci in range(2, len(col_tiles) - 1):
                    _, bc0, bcw = col_tiles[bci]
                    nc.gpsimd.dma_start(
                        out=b_sb[:, bc0 : bc0 + bcw],
                        in_=b_d[:, bc0 : bc0 + bcw],
                    )
                for half, (pst, nsx) in enumerate(((psA, nsA), (psB, nsB))):
                    for m in range(M_TILES):
                        yt = ypool.tile(
                            [128, 512], F32, name=f"ytF_{half}_{m}", tag="yt"
                        )
                        if first_tile_bias_matmul:
                            if m % 2 == 0:
                                nc.scalar.activation(
                                    yt[:], pst[m][:],
                                    mybir.ActivationFunctionType.Relu,
                                )
                            else:
                                nc.vector.tensor_scalar_max(
                                    yt[:], pst[m][:], 0.0
                                )
                        dma_eng = (nc.sync, nc.gpsimd, nc.scalar, nc.gpsimd)[m]
                        dma_eng.dma_start(
                            out=y_d[m * 128 : (m + 1) * 128, nsx], in_=yt[:]
                        )
                remaining = list(enumerate(col_tiles))[2:]
            else:
                remaining = list(enumerate(col_tiles))

            pair_cache = {}
            for ci, (n, c0, cw) in remaining:
                ns = slice(c0, c0 + cw)
                if last_m_outer and ci == len(col_tiles) - 1 and cw == 512:
                    # m-outer final tile: each batch-tile's accumulation
                    # finishes early so its relu+store overlaps the
                    # remaining matmuls; only one chain is left in the tail
                    mt_tiles = []
                    for k in range(K_TILES):
                        mtl = mlast_pool.tile(
                            [128, 512], BF16, name=f"mtl_{k}", tag="mtl"
                        )
                        nc.sync.dma_start(
                            out=mtl[:], in_=mw_d[k * 128 : (k + 1) * 128, ns]
                        )
                        mt_tiles.append(mtl)
                    for m in range(M_TILES):
                        psl = pspool.tile(
                            [128, 512], F32, name=f"ps_{ci}_{m}", tag="ps"
                        )
                        nc.tensor.matmul(
                            psl[:], ones_sb[:, 0:128], b0_sb[0:1, ns],
                            start=True, stop=False,
                        )
                        for k in range(K_TILES):
                            nc.tensor.matmul(
                                psl[:],
                                xt_sb[:, k, m * 128 : (m + 1) * 128],
                                mt_tiles[k][:],
                                start=False,
                                stop=(k == K_TILES - 1),
                            )
                        yt = ypool.tile(
                            [128, 512], F32, name=f"yt_{ci}_{m}", tag="yt"
                        )
                        nc.scalar.activation(
                            yt[:], psl[:], mybir.ActivationFunctionType.Relu
                        )
                        emit_store(ci, m, ns, cw, psl, yt)
                    continue
                ps = [
                    pspool.tile([128, 512], F32, name=f"ps_{ci}_{m}", tag="ps")
                    for m in range(M_TILES)
                ]
                mts = {}
                if ci == 0 and prefetch_mt:
                    for k in range(prefetch_mt):
                        load_xt(k)
                        mt = mpool.tile(
                            [128, 512], BF16, name=f"mt_{ci}_{k}", tag="mt"
                        )
                        nc.sync.dma_start(
                            out=mt[:, :cw], in_=mw_d[k * 128 : (k + 1) * 128, ns]
                        )
                        mts[k] = mt
                # first col-tile: bias matmuls fill the initial DMA wait and
                # warm the PE clock; last col-tile: they make the tail
                # evacuation a single relu op instead of add+relu
                bias_by_matmul = bias_mode == "matmul" or (
                    bias_mode == "evac"
                    and (
                        (ci == 0 and first_tile_bias_matmul)
                        or ci == len(col_tiles) - 1
                    )
                )
                if bias_by_matmul:
                    # out[128,cw] = ones[1,128].T @ b[1,cw]
                    for m in range(M_TILES):
                        nc.tensor.matmul(
                            ps[m][:, :cw], ones_sb[:, 0:128], b0_sb[0:1, ns],
                            start=True, stop=False,
                        )
                for k in range(K_TILES):
                    if k in mts:
                        mt = mts[k]
                    elif mt_pair and not split_last_n:
                        # one [128,1024] load serves this n-tile and the next
                        if ci == 0:
                            load_xt(k)
                        if n % 2 == 0:
                            mt2 = mpool.tile(
                                [128, 1024], BF16, name=f"mt2_{n // 2}_{k}",
                                tag="mt",
                            )
                            nc.sync.dma_start(
                                out=mt2[:],
                                in_=mw_d[k * 128 : (k + 1) * 128, c0 : c0 + 1024],
                            )
                            pair_cache[k] = mt2
                            mt = mt2[:, 0:512]
                        else:
                            mt = pair_cache[k][:, 512:1024]
                    else:
                        if ci == 0:
                            load_xt(k)
                        mt = mpool.tile(
                            [128, 512], BF16, name=f"mt_{ci}_{k}", tag="mt"
                        )
                        nc.sync.dma_start(
                            out=mt[:, :cw], in_=mw_d[k * 128 : (k + 1) * 128, ns]
                        )
                    for m in range(M_TILES):
                        nc.tensor.matmul(
                            ps[m][:, :cw],
                            xt_sb[:, k, m * 128 : (m + 1) * 128],
                            mt[:, :cw],
                            start=(not bias_by_matmul and k == 0),
                            stop=(k == K_TILES - 1),
                        )
                if bias_mode == "evac" and ci == 0:
                    # replicated-bias slices for the middle tiles, queued on
                    # the single SWDGE FIFO *behind* the whole xt stream:
                    # they can't start until n=0's critical loads are done,
                    # and land long before their first use (~2nd tile's
                    # evacuation)
                    nc.gpsimd.dma_start(out=b0_sb[:, 1024:], in_=b_d[0:1, 1024:])
                    for bci in range(1, len(col_tiles) - 1):
                        _, bc0, bcw = col_tiles[bci]
                        nc.gpsimd.dma_start(
                            out=b_sb[:, bc0 : bc0 + bcw],
                            in_=b_d[:, bc0 : bc0 + bcw],
                        )
                for m in range(M_TILES):
                    yt = ypool.tile([128, 512], F32, name=f"yt_{ci}_{m}", tag="yt")
                    if bias_mode == "evac" and not bias_by_matmul:
                        nc.vector.tensor_tensor(
                            yt[:, :cw], ps[m][:, :cw], b_sb[:, ns],
                            op=mybir.AluOpType.add,
                        )
                        nc.scalar.activation(
                            yt[:, :cw], yt[:, :cw],
                            mybir.ActivationFunctionType.Relu,
                        )
                    elif m % 2 == 0:
                        nc.scalar.activation(
                            yt[:, :cw], ps[m][:, :cw],
                            mybir.ActivationFunctionType.Relu,
                        )
                    else:
                        nc.vector.tensor_scalar_max(yt[:, :cw], ps[m][:, :cw], 0.0)
                    emit_store(ci, m, ns, cw, ps[m], yt)
    nc.compile()
    return nc


def _build_module_featpart(mpool_bufs: int = 8):
    """M-stationary layout: PSUM holds yT [feat(128-part), batch(512)].

    out = mtT.T @ xt: lhsT = a [128,128] column block of the M tile,
    rhs = the resident x^T chunk. The bias is then per-PARTITION, so it
    fuses into the relu on either evacuation engine as a single op
    (ACT: relu(psum*1 + bias); DVE: (psum add bias) max 0). No bias
    matmuls, no replicated-bias input. Output is y^T; the host
    transposes it back.
    """
    nc = bacc.Bacc("TRN2", target_bir_lowering=False, debug=False, num_devices=N_CORES)
    xt_d = nc.dram_tensor("xt", [D, BL], BF16, kind="ExternalInput")
    mw_d = nc.dram_tensor("mw", [D, D], BF16, kind="ExternalInput")
    # bias pre-arranged on host as [128, D//128]: column f holds
    # b[f*128:(f+1)*128] across partitions
    b_d = nc.dram_tensor("bv", [128, D // 128], F32, kind="ExternalInput")
    y_d = nc.dram_tensor("y", [D, BL], F32, kind="ExternalOutput")

    with tile.TileContext(nc) as tc:
        with (
            tc.tile_pool(name="const", bufs=1) as cpool,
            tc.tile_pool(name="mpool", bufs=mpool_bufs) as mpool,
            tc.tile_pool(name="ypool", bufs=4) as ypool,
            tc.tile_pool(name="pspool", bufs=8, space="PSUM") as pspool,
        ):
            xt_sb = cpool.tile([128, K_TILES, BL], BF16)
            b_sb = cpool.tile([128, D // 128], F32)
            nc.sync.dma_start(out=b_sb[:, :], in_=b_d[:, :])

            def load_xt(k):
                eng = nc.sync if (k == 0 or k >= 28) else nc.gpsimd
                eng.dma_start(
                    out=xt_sb[:, k, :], in_=xt_d[k * 128 : (k + 1) * 128, :]
                )

            n_blocks = D // 512
            for nb in range(n_blocks):
                ns = slice(nb * 512, (nb + 1) * 512)
                ps = [
                    pspool.tile([128, 512], F32, name=f"ps_{nb}_{fl}", tag="ps")
                    for fl in range(4)
                ]
                for k in range(K_TILES):
                    if nb == 0:
                        load_xt(k)
                    mt = mpool.tile([128, 512], BF16, name=f"mt_{nb}_{k}", tag="mt")
                    nc.sync.dma_start(
                        out=mt[:], in_=mw_d[k * 128 : (k + 1) * 128, ns]
                    )
                    for fl in range(4):
                        nc.tensor.matmul(
                            ps[fl][:],
                            mt[:, fl * 128 : (fl + 1) * 128],
                            xt_sb[:, k, :],
                            start=(k == 0),
                            stop=(k == K_TILES - 1),
                        )
                for fl in range(4):
                    f = nb * 4 + fl
                    yt = ypool.tile([128, 512], F32, name=f"yt_{nb}_{fl}", tag="yt")
                    if fl % 2 == 0:
                        nc.scalar.activation(
                            yt[:], ps[fl][:],
                            mybir.ActivationFunctionType.Relu,
                            bias=b_sb[:, f : f + 1],
                            scale=1.0,
                        )
                    else:
                        nc.vector.tensor_scalar(
                            yt[:], ps[fl][:],
                            b_sb[:, f : f + 1], 0.0,
                            mybir.AluOpType.add, mybir.AluOpType.max,
                        )
                    if nb == n_blocks - 1:
                        dma_eng = (nc.sync, nc.gpsimd, nc.scalar, nc.sync)[fl]
                    else:
                        dma_eng = (nc.sync, nc.gpsimd, nc.scalar, nc.gpsimd)[fl]
                    dma_eng.dma_start(
                        out=y_d[f * 128 : (f + 1) * 128, :], in_=yt[:]
                    )
    nc.compile()
    return nc


def _materialize_dense(core0, core1, core2, core3) -> np.ndarray:
    """M[(a0,a1,a2,a3),(b0,b1,b2,b3)] from TT cores [r,a,b,q], row-major."""
    t = np.asarray(core0, np.float32).reshape(8, 8, 16)        # a0,b0,r1
    t = np.tensordot(t, np.asarray(core1, np.float32), axes=([2], [0]))
    # a0,b0,a1,b1,r2
    t = np.tensordot(t, np.asarray(core2, np.float32), axes=([4], [0]))
    # a0,b0,a1,b1,a2,b2,r3
    t = np.tensordot(t, np.asarray(core3, np.float32), axes=([6], [0]))[..., 0]
    # a0,b0,a1,b1,a2,b2,a3,b3
    return np.ascontiguousarray(
        t.transpose(0, 2, 4, 6, 1, 3, 5, 7).reshape(D, D)
    )


_module_cache: list = []


def kernel(x, core0, core1, core2, core3, b):
    bf = ml_dtypes.bfloat16
    M = _materialize_dense(core0, core1, core2, core3)
    Mb = M.astype(bf)
    # bias replicated across the 128 PSUM partitions for the evacuation add
    bb = np.ascontiguousarray(
        np.broadcast_to(np.asarray(b, np.float32).astype(bf), (128, D))
    )
    x = np.asarray(x, np.float32)

    in_maps = []
    for c in range(N_CORES):
        xt = np.ascontiguousarray(x[c * BL : (c + 1) * BL].T).astype(bf)
        in_maps.append({"xt": xt, "mw": Mb, "bv": bb})

    if not _module_cache:
        _module_cache.append(_build_module())
    nc = _module_cache[0]
    res = run_bass_kernel_spmd(nc, in_maps, core_ids=list(range(N_CORES)))
    return np.concatenate([res.results[c]["y"] for c in range(N_CORES)], axis=0)



# revision 3
# speedup vs baseline: 1.7119x; 1.7119x over previous
"""TT-dense layer (BayesKerasDense): y = relu(x @ M + b), M given as a
4-core tensor-train.

Strategy: materialize the dense M = TT(core0..core3) on the host (cheap) and
run a data-parallel dense matmul on 8 NeuronCores. The matmul runs in fp8
(e4m3) with MatmulPerfMode.DoubleRow: one PE instruction contracts TWO
128-deep k-tiles at 0.5 cycles/row, i.e. 4x bf16 throughput.

fp8 alone is too lossy (measured 3.3e-2 max-rel vs the 2e-2 gate), so we use
a residual-folded two-pass scheme (Karatsuba-style scale folding):

    xh = q8(x),  xl = x - xh          Mh = q8(M),  Ml = M - Mh
    B  = q8(s*xh + xl)                Q  = q8(Mh + Ml/s)        (s = 1/8)
    y  = (1-s)*(xh @ Mh) + B @ Q  + b
       = x @ M  - xl@Ml*(1/s-1)  + O(s*eps)   [measured 6.1e-3 max-rel]

The quantization scales satisfy sB*sQ = sx*sm/(1-s) so BOTH passes carry the
same final coefficient and can accumulate into a single PSUM bank; the bias
is folded in as a K=1 fp8 matmul (ones*o @ brow), so evacuation is a single
relu-with-scale op (alternating ACT/DVE). Per core: PE ~= 2 fp8-DR passes
(~110us), DMA ~= 40MB fp8/fp16 (~116us), fully overlapped.
"""

import sys

import numpy as np
import ml_dtypes

try:
    import concourse.bacc as bacc
except ImportError:  # fallback for environments without the site hook
    sys.path.insert(0, "/opt/trn_rl_repo")
    import concourse.bacc as bacc
import concourse.mybir as mybir
import concourse.tile as tile
from concourse.bass_utils import run_bass_kernel_spmd

N_CORES = 8
B = 4096           # global batch
BL = B // N_CORES  # per-core batch (512)
D = 4096           # n_in == n_out

NT = D // 512      # 8 column tiles of 512
JT = D // 256      # 16 k-pair chunks (each covers 256 of K via DoubleRow)
ZT = BL // 128     # 4 batch slices of 128
CH = 4             # j's per M-side DMA chunk
NCH = JT // CH     # 4 chunks per (n-tile, matrix)

S_SPLIT = 0.125
SX = 32.0
SM = 1024.0
G_EVAC = (1.0 - S_SPLIT) / (SX * SM)   # final PSUM scale (exact in fp32)
SB = 224.0
SQ = 1.0 / (G_EVAC * SB)

F8 = ml_dtypes.float8_e4m3
FP8 = mybir.dt.float8e4
F16 = mybir.dt.float16
F32 = mybir.dt.float32


def _build_module(warmup_mms: int = 10):
    nc = bacc.Bacc("TRN2", target_bir_lowering=False, debug=False,
                   num_devices=N_CORES)
    xh_d = nc.dram_tensor("xh", [128, JT, 2, BL], FP8, kind="ExternalInput")
    bb_d = nc.dram_tensor("bb", [128, JT, 2, BL], FP8, kind="ExternalInput")
    mh_d = nc.dram_tensor("mh", [128, NT, JT, 2, 512], FP8, kind="ExternalInput")
    qq_d = nc.dram_tensor("qq", [128, NT, JT, 2, 512], FP8, kind="ExternalInput")
    # const row: cols [0:128) = bias-matmul lhs value o, [128:128+D) = brow
    cr_d = nc.dram_tensor("cr", [1, 128 + D], FP8, kind="ExternalInput")
    y_d = nc.dram_tensor("y", [BL, D], F16, kind="ExternalOutput")

    with tile.TileContext(nc) as tc:
        with (
            tc.tile_pool(name="const", bufs=1) as cpool,
            tc.tile_pool(name="mpool", bufs=16) as mpool,
            tc.tile_pool(name="ypool", bufs=4) as ypool,
            tc.tile_pool(name="pspool", bufs=8, space="PSUM") as pspool,
        ):
            cr_sb = cpool.tile([1, 128 + D], FP8)
            nc.scalar.dma_start(out=cr_sb[:], in_=cr_d[:])

            # warmup matmuls with no DMA deps: keep the PE busy from t~0 so
            # the p-state ramp (slow clock for the first ~3us of busy time)
            # burns off before the real work arrives
            if warmup_mms:
                w1 = cpool.tile([1, 128], FP8)
                w2 = cpool.tile([1, 512], FP8)
                nc.vector.memset(w1[:], 1.0)
                nc.vector.memset(w2[:], 1.0)
                for w in range(warmup_mms):
                    wps = pspool.tile([128, 512], F32, name=f"wps_{w}", tag="ps")
                    nc.tensor.matmul(wps[:], w1[:], w2[:], start=True, stop=True)

            # x-side operands, resident in SBUF (16KB/partition each)
            xh_sb = cpool.tile([128, JT, 2, BL], FP8)
            bb_sb = cpool.tile([128, JT, 2, BL], FP8)

            def load_xside(dst, src, c):
                nc.sync.dma_start(
                    out=dst[:, c * CH:(c + 1) * CH, :, :],
                    in_=src[:, c * CH:(c + 1) * CH, :, :],
                )

            mh_tiles = {}
            qq_tiles = {}

            def load_mside(tiles, src, n, c, tag):
                t = mpool.tile([128, CH, 2, 512], FP8, name=f"{tag}_{n}_{c}",
                               tag="mt")
                nc.sync.dma_start(
                    out=t[:], in_=src[:, n, c * CH:(c + 1) * CH, :, :]
                )
                tiles[(n, c)] = t

            for n in range(NT):
                # DMA emission order == sync-queue order. For n=0 interleave
                # the x-side loads so the first DR matmuls unblock early.
                for c in range(NCH):
                    if n == 0:
                        load_xside(xh_sb, xh_d, c)
                    load_mside(mh_tiles, mh_d, n, c, "mh")
                for c in range(NCH):
                    if n == 0:
                        load_xside(bb_sb, bb_d, c)
                    load_mside(qq_tiles, qq_d, n, c, "qq")

                ns = slice(128 + n * 512, 128 + (n + 1) * 512)
                ps = [
                    pspool.tile([128, 512], F32, name=f"ps_{n}_{z}", tag="ps")
                    for z in range(ZT)
                ]
                for z in range(ZT):
                    nc.tensor.matmul(
                        ps[z][:], cr_sb[:, 0:128], cr_sb[:, ns],
                        start=True, stop=False,
                    )
                for half, (xt, tiles) in enumerate(
                    ((xh_sb, mh_tiles), (bb_sb, qq_tiles))
                ):
                    for j in range(JT):
                        mt = tiles[(n, j // CH)]
                        for z in range(ZT):
                            nc.tensor.matmul(
                                ps[z][:],
                                xt[:, j, :, z * 128:(z + 1) * 128],
                                mt[:, j % CH, :, :],
                                start=False,
                                stop=(half == 1 and j == JT - 1),
                                perf_mode=mybir.MatmulPerfMode.DoubleRow,
                            )
                for z in range(ZT):
                    yt = ypool.tile([128, 512], F16, name=f"yt_{n}_{z}",
                                    tag="yt")
                    if z % 2 == 0:
                        nc.scalar.activation(
                            yt[:], ps[z][:],
                            mybir.ActivationFunctionType.Relu, scale=G_EVAC,
                        )
                    else:
                        nc.vector.tensor_scalar(
                            yt[:], ps[z][:], G_EVAC, 0.0,
                            mybir.AluOpType.mult, mybir.AluOpType.max,
                        )
                    st_eng = (nc.scalar, nc.gpsimd, nc.scalar, nc.gpsimd)[z]
                    st_eng.dma_start(
                        out=y_d[z * 128:(z + 1) * 128, n * 512:(n + 1) * 512],
                        in_=yt[:],
                    )
    nc.compile()
    return nc


def _materialize_dense(core0, core1, core2, core3) -> np.ndarray:
    """M[(a0,a1,a2,a3),(b0,b1,b2,b3)] from TT cores [r,a,b,q], row-major."""
    t = np.asarray(core0, np.float64).reshape(8, 8, 16)        # a0,b0,r1
    t = np.tensordot(t, np.asarray(core1, np.float64), axes=([2], [0]))
    t = np.tensordot(t, np.asarray(core2, np.float64), axes=([4], [0]))
    t = np.tensordot(t, np.asarray(core3, np.float64), axes=([6], [0]))[..., 0]
    return np.ascontiguousarray(
        t.transpose(0, 2, 4, 6, 1, 3, 5, 7).reshape(D, D)
    )


def _f8(a):
    return np.asarray(a, np.float32).astype(F8)


def _pack_kmajor(a, ncols):
    """[K, ncols] -> [128, K//256, 2, ncols] with k = j*256 + i*128 + p."""
    return np.ascontiguousarray(
        a.reshape(JT, 2, 128, ncols).transpose(2, 0, 1, 3)
    )


_module_cache: list = []


def kernel(x, core0, core1, core2, core3, b):
    M = _materialize_dense(core0, core1, core2, core3)
    x = np.asarray(x, np.float64)
    b64 = np.asarray(b, np.float64)

    s = S_SPLIT
    Mh8 = _f8(SM * M)
    Mh = Mh8.astype(np.float64) / SM
    Q8 = _f8(SQ * (Mh + (M - Mh) / s))

    # [128, NT, JT, 2, 512] fp8 streams
    def pack_mside(m8):
        return np.ascontiguousarray(
            m8.reshape(JT, 2, 128, NT, 512).transpose(2, 3, 0, 1, 4)
        )

    mh_p = pack_mside(Mh8)
    qq_p = pack_mside(Q8)

    # bias: y += g * (o * brow); pick the fp8 value o minimizing bias error
    o_grid = np.unique(np.abs(
        np.arange(16, 241, dtype=np.float32).astype(F8).astype(np.float32)))
    best = None
    for o in o_grid:
        if o <= 0:
            continue
        beta = _f8(b64 / (G_EVAC * o))
        err = np.abs(o * G_EVAC * beta.astype(np.float64) - b64).max()
        if best is None or err < best[0]:
            best = (err, float(o), beta)
    _, o_val, beta8 = best
    cr = np.zeros(128 + D, F8)
    cr[0:128] = np.float32(o_val).astype(F8)
    cr[128:] = beta8
    cr = cr.reshape(1, 128 + D)

    in_maps = []
    for c in range(N_CORES):
        xc = x[c * BL:(c + 1) * BL]                   # [BL, D]
        xh8 = _f8(SX * xc)
        xh = xh8.astype(np.float64) / SX
        B8 = _f8(SB * (s * xh + (xc - xh)))
        in_maps.append({
            "xh": _pack_kmajor(xh8.T, BL),
            "bb": _pack_kmajor(B8.T, BL),
            "mh": mh_p,
            "qq": qq_p,
            "cr": cr,
        })

    if not _module_cache:
        _module_cache.append(_build_module())
    nc = _module_cache[0]
    res = run_bass_kernel_spmd(nc, in_maps, core_ids=list(range(N_CORES)))
    out = np.concatenate(
        [res.results[c]["y"].astype(np.float32) for c in range(N_CORES)],
        axis=0,
    )
    return out


# revision 27
# speedup vs baseline: 1.7986x; 1.0507x over previous
"""TT-dense layer (BayesKerasDense): y = relu(x @ M + b), M given as a
4-core tensor-train.

Strategy: materialize the dense M = TT(core0..core3) on the host (cheap) and
run a data-parallel dense matmul on 8 NeuronCores. The matmul runs in fp8
(e4m3) with MatmulPerfMode.DoubleRow: one PE instruction contracts TWO
128-deep k-tiles at 0.5 cycles/row, i.e. 4x bf16 throughput.

fp8 alone is too lossy (measured 3.3e-2 max-rel vs the 2e-2 gate), so we use
a residual-folded two-pass scheme (Karatsuba-style scale folding):

    xh = q8(x),  xl = x - xh          Mh = q8(M),  Ml = M - Mh
    B  = q8(s*xh + xl)                Q  = q8(Mh + Ml/s)        (s = 1/8)
    y  = (1-s)*(xh @ Mh) + B @ Q  + b
       = x @ M  - xl@Ml*(1/s-1)  + O(s*eps)   [measured 6.1e-3 max-rel]

The quantization scales satisfy sB*sQ = sx*sm/(1-s) so BOTH passes carry the
same final coefficient and can accumulate into a single PSUM bank; the bias
is folded in as a K=1 fp8 matmul (ones*o @ brow), so evacuation is a single
relu-with-scale op (alternating ACT/DVE). Per core: PE ~= 2 fp8-DR passes
(~110us), DMA ~= 40MB fp8/fp16 (~116us), fully overlapped.
"""

import sys

import numpy as np
import ml_dtypes

try:
    import concourse.bacc as bacc
except ImportError:  # fallback for environments without the site hook
    sys.path.insert(0, "/opt/trn_rl_repo")
    import concourse.bacc as bacc
import concourse.mybir as mybir
import concourse.tile as tile
from concourse.bass_utils import run_bass_kernel_spmd

N_CORES = 8
B = 4096           # global batch
BL = B // N_CORES  # per-core batch (512)
D = 4096           # n_in == n_out

NT = D // 512      # 8 column tiles of 512
JT = D // 256      # 16 k-pair chunks (each covers 256 of K via DoubleRow)
ZT = BL // 128     # 4 batch slices of 128
CH = 4             # j's per M-side DMA chunk
NCH = JT // CH     # 4 chunks per (n-tile, matrix)

S_SPLIT = 0.125
SX = 32.0
SM = 1024.0
G_EVAC = (1.0 - S_SPLIT) / (SX * SM)   # final PSUM scale (exact in fp32)
SB = 224.0
SQ = 1.0 / (G_EVAC * SB)

F8 = ml_dtypes.float8_e4m3
FP8 = mybir.dt.float8e4
F16 = mybir.dt.float16
F32 = mybir.dt.float32


def _build_module(warmup_mms: int = 8):
    nc = bacc.Bacc("TRN2", target_bir_lowering=False, debug=False,
                   num_devices=N_CORES)
    xh_d = nc.dram_tensor("xh", [128, JT, 2, BL], FP8, kind="ExternalInput")
    bb_d = nc.dram_tensor("bb", [128, JT, 2, BL], FP8, kind="ExternalInput")
    mh_d = nc.dram_tensor("mh", [128, NT, JT, 2, 512], FP8, kind="ExternalInput")
    qq_d = nc.dram_tensor("qq", [128, NT, JT, 2, 512], FP8, kind="ExternalInput")
    # const rows, one per DoubleRow k-slot: [o(128) | brow_half(D)] each.
    # The bias matmul is itself a DoubleRow op: each slot adds o*brow_half.
    cr_d = nc.dram_tensor("cr", [1, 2, 128 + D], FP8, kind="ExternalInput")
    y_d = nc.dram_tensor("y", [BL, D], F16, kind="ExternalOutput")

    with tile.TileContext(nc) as tc:
        with (
            tc.tile_pool(name="const", bufs=1) as cpool,
            tc.tile_pool(name="mpool", bufs=16) as mpool,
            tc.tile_pool(name="ypool", bufs=4) as ypool,
            tc.tile_pool(name="tmppool", bufs=4) as tmppool,
            tc.tile_pool(name="pspool", bufs=8, space="PSUM") as pspool,
        ):
            cr_sb = cpool.tile([1, 2, 128 + D], FP8)
            nc.scalar.dma_start(out=cr_sb[:], in_=cr_d[:])

            # warmup matmuls with no DMA deps: keep the PE busy through the
            # DMA-bound startup (x-side + first M tiles ~ 4MB) and burn off
            # the p-state ramp (slow PE clock for the first ~3us busy)
            if warmup_mms:
                wt = cpool.tile([1, 640], FP8)
                nc.vector.memset(wt[:], 1.0)
                for w in range(warmup_mms):
                    wps = pspool.tile([128, 512], F32, name=f"wps_{w}", tag="ps")
                    nc.tensor.matmul(wps[:], wt[:, 0:128], wt[:, 128:640],
                                     start=True, stop=True)

            # x-side operands, resident in SBUF (16KB/partition each)
            xh_sb = cpool.tile([128, JT, 2, BL], FP8)
            bb_sb = cpool.tile([128, JT, 2, BL], FP8)

            def load_xside(dst, src, c):
                nc.sync.dma_start(
                    out=dst[:, c * CH:(c + 1) * CH, :, :],
                    in_=src[:, c * CH:(c + 1) * CH, :, :],
                )

            mh_tiles = {}
            qq_tiles = {}

            def load_mside(tiles, src, n, c, tag):
                t = mpool.tile([128, CH, 2, 512], FP8, name=f"{tag}_{n}_{c}",
                               tag="mt")
                nc.sync.dma_start(
                    out=t[:], in_=src[:, n, c * CH:(c + 1) * CH, :, :]
                )
                tiles[(n, c)] = t

            ps_tiles = {}
            spill_sb = {}

            def emit_p1(n):
                # opens tile n's groups (bias matmul), runs the Mh pass,
                # closes the groups (stop on the last j)
                ns = slice(128 + n * 512, 128 + (n + 1) * 512)
                ps_tiles[n] = {}
                for z in range(ZT):
                    ps = pspool.tile([128, 512], F32, name=f"ps_{n}_{z}",
                                     tag="ps")
                    ps_tiles[n][z] = ps
                    nc.tensor.matmul(
                        ps[:], cr_sb[:, :, 0:128], cr_sb[:, :, ns],
                        start=True, stop=False,
                        perf_mode=mybir.MatmulPerfMode.DoubleRow,
                    )
                for j in range(JT):
                    msl = mh_tiles[(n, j // CH)][:, j % CH, :, :]
                    for z in range(ZT):
                        nc.tensor.matmul(
                            ps_tiles[n][z][:],
                            xh_sb[:, j, :, z * 128:(z + 1) * 128],
                            msl,
                            start=False, stop=(j == JT - 1),
                            perf_mode=mybir.MatmulPerfMode.DoubleRow,
                        )

            def emit_spill(n):
                # copy tile n's closed P1 partials PSUM -> SBUF, freeing the
                # banks; runs on the idle ACT/DVE engines during later P1s
                for z in range(ZT):
                    sp = cpool.tile([128, 512], F32, name=f"spill_{n}_{z}")
                    spill_sb[(n, z)] = sp
                    if z % 2 == 0:
                        nc.scalar.copy(sp[:], ps_tiles[n][z][:])
                    else:
                        nc.vector.tensor_scalar_add(sp[:], ps_tiles[n][z][:],
                                                    0.0)

            def emit_p2_evac(n):
                # reopened groups: P2 fresh-starts in a recycled bank; the
                # evacuation adds the spilled P1 partial back, then relu*g
                last = n == NT - 1
                ps2 = {}
                for z in range(ZT):
                    if last and z == ZT - 1:
                        # final slice split into two half-groups (each its own
                        # bank): the first half closes early so its evac+store
                        # chain overlaps the second half's matmuls
                        ps2[(z, 0)] = pspool.tile([128, 512], F32,
                                                  name=f"ps2_{n}_{z}a",
                                                  tag="ps")[:, 0:256]
                        ps2[(z, 1)] = pspool.tile([128, 512], F32,
                                                  name=f"ps2_{n}_{z}b",
                                                  tag="ps")[:, 0:256]
                    else:
                        ps2[z] = pspool.tile([128, 512], F32,
                                             name=f"ps2_{n}_{z}", tag="ps")
                for z in range(ZT):
                    rows = slice(z * 128, (z + 1) * 128)
                    if last and z == ZT - 1:
                        tmp = tmppool.tile([128, 512], F32,
                                           name=f"tmp_{n}_{z}", tag="tmp")
                        yt = ypool.tile([128, 512], F16, name=f"yt_{n}_{z}",
                                        tag="yt")
                        for h, eng in ((0, nc.sync), (1, nc.scalar)):
                            cols = slice(h * 256, (h + 1) * 256)
                            for j in range(JT):
                                mt = qq_tiles[(n, j // CH)]
                                nc.tensor.matmul(
                                    ps2[(z, h)][:],
                                    bb_sb[:, j, :, z * 128:(z + 1) * 128],
                                    mt[:, j % CH, :, cols],
                                    start=(j == 0), stop=(j == JT - 1),
                                    perf_mode=mybir.MatmulPerfMode.DoubleRow,
                                )
                            nc.vector.tensor_tensor(
                                tmp[:, cols], ps2[(z, h)][:],
                                spill_sb[(n, z)][:, cols],
                                op=mybir.AluOpType.add,
                            )
                            nc.vector.tensor_scalar(
                                yt[:, cols], tmp[:, cols], G_EVAC, 0.0,
                                mybir.AluOpType.mult, mybir.AluOpType.max,
                            )
                            eng.dma_start(
                                out=y_d[rows, n * 512 + h * 256:
                                        n * 512 + (h + 1) * 256],
                                in_=yt[:, cols],
                            )
                        continue
                    for j in range(JT):
                        mt = qq_tiles[(n, j // CH)]
                        nc.tensor.matmul(
                            ps2[z][:],
                            bb_sb[:, j, :, z * 128:(z + 1) * 128],
                            mt[:, j % CH, :, :],
                            start=(j == 0), stop=(j == JT - 1),
                            perf_mode=mybir.MatmulPerfMode.DoubleRow,
                        )
                    tmp = tmppool.tile([128, 512], F32, name=f"tmp_{n}_{z}",
                                       tag="tmp")
                    yt = ypool.tile([128, 512], F16, name=f"yt_{n}_{z}",
                                    tag="yt")
                    nc.vector.tensor_tensor(
                        tmp[:], ps2[z][:], spill_sb[(n, z)][:],
                        op=mybir.AluOpType.add,
                    )
                    nc.scalar.activation(
                        yt[:], tmp[:],
                        mybir.ActivationFunctionType.Relu, scale=G_EVAC,
                    )
                    if last:
                        st_eng = (nc.scalar, nc.sync, nc.scalar, nc.sync)[z]
                    else:
                        st_eng = (nc.scalar, nc.gpsimd, nc.scalar,
                                  nc.gpsimd)[z]
                    st_eng.dma_start(
                        out=y_d[rows, n * 512:(n + 1) * 512], in_=yt[:],
                    )

            # Two-phase schedule with PSUM spilling. Phase 1: all eight Mh
            # passes back-to-back (the spill breaks the 8-bank pipeline-depth
            # wall); DMA streams xh, mh0..mh7 with no x-side stall. Phase 2:
            # all Q passes, consuming bb, qq0..qq7; evacuation folds the
            # spilled partial back in. Both phases are PE-bound vs their DMA
            # streams, so the PE runs stall-free after the initial fill.
            for c in range(NCH):
                load_xside(xh_sb, xh_d, c)
                load_mside(mh_tiles, mh_d, 0, c, "mh")
            emit_p1(0)
            for n in range(1, NT):
                for c in range(NCH):
                    load_mside(mh_tiles, mh_d, n, c, "mh")
                emit_p1(n)
                emit_spill(n - 1)
            emit_spill(NT - 1)
            for c in range(NCH):
                load_xside(bb_sb, bb_d, c)
            for n in range(NT):
                for c in range(NCH):
                    load_mside(qq_tiles, qq_d, n, c, "qq")
                emit_p2_evac(n)
    nc.compile()
    return nc


def _materialize_dense(core0, core1, core2, core3) -> np.ndarray:
    """M[(a0,a1,a2,a3),(b0,b1,b2,b3)] from TT cores [r,a,b,q], row-major."""
    t = np.asarray(core0, np.float64).reshape(8, 8, 16)        # a0,b0,r1
    t = np.tensordot(t, np.asarray(core1, np.float64), axes=([2], [0]))
    t = np.tensordot(t, np.asarray(core2, np.float64), axes=([4], [0]))
    t = np.tensordot(t, np.asarray(core3, np.float64), axes=([6], [0]))[..., 0]
    return np.ascontiguousarray(
        t.transpose(0, 2, 4, 6, 1, 3, 5, 7).reshape(D, D)
    )


def _f8(a):
    return np.asarray(a, np.float32).astype(F8)


def _pack_kmajor(a, ncols):
    """[K, ncols] -> [128, K//256, 2, ncols] with k = j*256 + i*128 + p."""
    return np.ascontiguousarray(
        a.reshape(JT, 2, 128, ncols).transpose(2, 0, 1, 3)
    )


_module_cache: list = []


def kernel(x, core0, core1, core2, core3, b):
    M = _materialize_dense(core0, core1, core2, core3)
    x = np.asarray(x, np.float64)
    b64 = np.asarray(b, np.float64)

    s = S_SPLIT
    Mh8 = _f8(SM * M)
    Mh = Mh8.astype(np.float64) / SM
    Q8 = _f8(SQ * (Mh + (M - Mh) / s))

    # [128, NT, JT, 2, 512] fp8 streams
    def pack_mside(m8):
        return np.ascontiguousarray(
            m8.reshape(JT, 2, 128, NT, 512).transpose(2, 3, 0, 1, 4)
        )

    mh_p = pack_mside(Mh8)
    qq_p = pack_mside(Q8)

    # bias: y += g * 2 * (o * brow_half), brow duplicated across the two
    # DoubleRow k-slots; pick the fp8 value o minimizing bias error
    o_grid = np.unique(np.abs(
        np.arange(16, 241, dtype=np.float32).astype(F8).astype(np.float32)))
    best = None
    for o in o_grid:
        if o <= 0:
            continue
        beta = _f8(b64 / (2.0 * G_EVAC * o))
        err = np.abs(2.0 * o * G_EVAC * beta.astype(np.float64) - b64).max()
        if best is None or err < best[0]:
            best = (err, float(o), beta)
    _, o_val, beta8 = best
    cr = np.zeros((2, 128 + D), F8)
    cr[:, 0:128] = np.float32(o_val).astype(F8)
    cr[0, 128:] = beta8
    cr[1, 128:] = beta8
    cr = cr.reshape(1, 2, 128 + D)

    in_maps = []
    for c in range(N_CORES):
        xc = x[c * BL:(c + 1) * BL]                   # [BL, D]
        xh8 = _f8(SX * xc)
        xh = xh8.astype(np.float64) / SX
        B8 = _f8(SB * (s * xh + (xc - xh)))
        in_maps.append({
            "xh": _pack_kmajor(xh8.T, BL),
            "bb": _pack_kmajor(B8.T, BL),
            "mh": mh_p,
            "qq": qq_p,
            "cr": cr,
        })

    if not _module_cache:
        _module_cache.append(_build_module())
    nc = _module_cache[0]
    res = run_bass_kernel_spmd(nc, in_maps, core_ids=list(range(N_CORES)))
    out = np.concatenate(
        [res.results[c]["y"].astype(np.float32) for c in range(N_CORES)],
        axis=0,
    )
    return out


# revision 37
# speedup vs baseline: 1.8017x; 1.0017x over previous
"""TT-dense layer (BayesKerasDense): y = relu(x @ M + b), M given as a
4-core tensor-train.

Strategy: materialize the dense M = TT(core0..core3) on the host (cheap) and
run a data-parallel dense matmul on 8 NeuronCores. The matmul runs in fp8
(e4m3) with MatmulPerfMode.DoubleRow: one PE instruction contracts TWO
128-deep k-tiles at 0.5 cycles/row, i.e. 4x bf16 throughput.

fp8 alone is too lossy (measured 3.3e-2 max-rel vs the 2e-2 gate), so we use
a residual-folded two-pass scheme (Karatsuba-style scale folding):

    xh = q8(x),  xl = x - xh          Mh = q8(M),  Ml = M - Mh
    B  = q8(s*xh + xl)                Q  = q8(Mh + Ml/s)        (s = 1/8)
    y  = (1-s)*(xh @ Mh) + B @ Q  + b
       = x @ M  - xl@Ml*(1/s-1)  + O(s*eps)   [measured 6.1e-3 max-rel]

The quantization scales satisfy sB*sQ = sx*sm/(1-s) so BOTH passes carry the
same final coefficient and can accumulate into a single PSUM bank; the bias
is folded in as a K=1 fp8 matmul (ones*o @ brow), so evacuation is a single
relu-with-scale op (alternating ACT/DVE). Per core: PE ~= 2 fp8-DR passes
(~110us), DMA ~= 40MB fp8/fp16 (~116us), fully overlapped.
"""

import sys

import numpy as np
import ml_dtypes

try:
    import concourse.bacc as bacc
except ImportError:  # fallback for environments without the site hook
    sys.path.insert(0, "/opt/trn_rl_repo")
    import concourse.bacc as bacc
import concourse.mybir as mybir
import concourse.tile as tile
from concourse.bass_utils import run_bass_kernel_spmd

N_CORES = 8
B = 4096           # global batch
BL = B // N_CORES  # per-core batch (512)
D = 4096           # n_in == n_out

NT = D // 512      # 8 column tiles of 512
JT = D // 256      # 16 k-pair chunks (each covers 256 of K via DoubleRow)
ZT = BL // 128     # 4 batch slices of 128
CH = 4             # j's per M-side DMA chunk
NCH = JT // CH     # 4 chunks per (n-tile, matrix)

S_SPLIT = 0.125
SX = 32.0
SM = 1024.0
G_EVAC = (1.0 - S_SPLIT) / (SX * SM)   # final PSUM scale (exact in fp32)
SB = 224.0
SQ = 1.0 / (G_EVAC * SB)

F8 = ml_dtypes.float8_e4m3
FP8 = mybir.dt.float8e4
F16 = mybir.dt.float16
F32 = mybir.dt.float32


def _build_module(warmup_mms: int = 8):
    nc = bacc.Bacc("TRN2", target_bir_lowering=False, debug=False,
                   num_devices=N_CORES)
    xh_d = nc.dram_tensor("xh", [128, JT, 2, BL], FP8, kind="ExternalInput")
    bb_d = nc.dram_tensor("bb", [128, JT, 2, BL], FP8, kind="ExternalInput")
    mh_d = nc.dram_tensor("mh", [128, NT, JT, 2, 512], FP8, kind="ExternalInput")
    qq_d = nc.dram_tensor("qq", [128, NT, JT, 2, 512], FP8, kind="ExternalInput")
    # const rows, one per DoubleRow k-slot: [o(128) | brow_half(D)] each.
    # The bias matmul is itself a DoubleRow op: each slot adds o*brow_half.
    cr_d = nc.dram_tensor("cr", [1, 2, 128 + D], FP8, kind="ExternalInput")
    # half-width repacks of tile 0's Mh and tile 7's Q (column halves
    # contiguous) for the startup/tail critical paths
    mh0h_d = nc.dram_tensor("mh0h", [128, 2, JT, 2, 256], FP8,
                            kind="ExternalInput")
    qq7h_d = nc.dram_tensor("qq7h", [128, 2, JT, 2, 256], FP8,
                            kind="ExternalInput")
    y_d = nc.dram_tensor("y", [BL, D], F16, kind="ExternalOutput")

    with tile.TileContext(nc) as tc:
        with (
            tc.tile_pool(name="const", bufs=1) as cpool,
            tc.tile_pool(name="mpool", bufs=16) as mpool,
            tc.tile_pool(name="ypool", bufs=4) as ypool,
            tc.tile_pool(name="tmppool", bufs=4) as tmppool,
            tc.tile_pool(name="pspool", bufs=8, space="PSUM") as pspool,
        ):
            cr_sb = cpool.tile([1, 2, 128 + D], FP8)
            nc.scalar.dma_start(out=cr_sb[:], in_=cr_d[:])

            # warmup matmuls with no DMA deps: keep the PE busy through the
            # DMA-bound startup (x-side + first M tiles ~ 4MB) and burn off
            # the p-state ramp (slow PE clock for the first ~3us busy)
            if warmup_mms:
                wt = cpool.tile([1, 640], FP8)
                nc.vector.memset(wt[:], 1.0)
                for w in range(warmup_mms):
                    wps = pspool.tile([128, 512], F32, name=f"wps_{w}", tag="ps")
                    nc.tensor.matmul(wps[:], wt[:, 0:128], wt[:, 128:640],
                                     start=True, stop=True)

            # x-side operands, resident in SBUF (16KB/partition each)
            xh_sb = cpool.tile([128, JT, 2, BL], FP8)
            bb_sb = cpool.tile([128, JT, 2, BL], FP8)

            def load_xside(dst, src, c):
                nc.sync.dma_start(
                    out=dst[:, c * CH:(c + 1) * CH, :, :],
                    in_=src[:, c * CH:(c + 1) * CH, :, :],
                )

            mh_tiles = {}
            qq_tiles = {}

            def load_mside(tiles, src, n, c, tag):
                t = mpool.tile([128, CH, 2, 512], FP8, name=f"{tag}_{n}_{c}",
                               tag="mt")
                nc.sync.dma_start(
                    out=t[:], in_=src[:, n, c * CH:(c + 1) * CH, :, :]
                )
                tiles[(n, c)] = t

            ps_tiles = {}
            ps_half = {}
            spill_sb = {}
            mh0h_tiles = {}
            qq7h_tiles = {}

            def emit_p1(n):
                # opens tile n's groups (bias matmul), runs the Mh pass,
                # closes the groups (stop on the last j)
                ns = slice(128 + n * 512, 128 + (n + 1) * 512)
                ps_tiles[n] = {}
                for z in range(ZT):
                    ps = pspool.tile([128, 512], F32, name=f"ps_{n}_{z}",
                                     tag="ps")
                    ps_tiles[n][z] = ps
                    nc.tensor.matmul(
                        ps[:], cr_sb[:, :, 0:128], cr_sb[:, :, ns],
                        start=True, stop=False,
                        perf_mode=mybir.MatmulPerfMode.DoubleRow,
                    )
                for j in range(JT):
                    msl = mh_tiles[(n, j // CH)][:, j % CH, :, :]
                    for z in range(ZT):
                        nc.tensor.matmul(
                            ps_tiles[n][z][:],
                            xh_sb[:, j, :, z * 128:(z + 1) * 128],
                            msl,
                            start=False, stop=(j == JT - 1),
                            perf_mode=mybir.MatmulPerfMode.DoubleRow,
                        )

            def emit_spill(n):
                # copy tile n's closed P1 partials PSUM -> SBUF, freeing the
                # banks; runs on the idle ACT/DVE engines during later P1s
                for z in range(ZT):
                    sp = cpool.tile([128, 512], F32, name=f"spill_{n}_{z}")
                    spill_sb[(n, z)] = sp
                    if z % 2 == 0:
                        nc.scalar.copy(sp[:], ps_tiles[n][z][:])
                    else:
                        nc.vector.tensor_scalar_add(sp[:], ps_tiles[n][z][:],
                                                    0.0)

            def emit_p2_evac(n):
                # reopened groups: P2 fresh-starts in a recycled bank; the
                # evacuation adds the spilled P1 partial back, then relu*g
                ps2 = {}
                for z in range(ZT):
                    ps2[z] = pspool.tile([128, 512], F32,
                                         name=f"ps2_{n}_{z}", tag="ps")
                for z in range(ZT):
                    rows = slice(z * 128, (z + 1) * 128)
                    for j in range(JT):
                        mt = qq_tiles[(n, j // CH)]
                        nc.tensor.matmul(
                            ps2[z][:],
                            bb_sb[:, j, :, z * 128:(z + 1) * 128],
                            mt[:, j % CH, :, :],
                            start=(j == 0), stop=(j == JT - 1),
                            perf_mode=mybir.MatmulPerfMode.DoubleRow,
                        )
                    tmp = tmppool.tile([128, 512], F32, name=f"tmp_{n}_{z}",
                                       tag="tmp")
                    yt = ypool.tile([128, 512], F16, name=f"yt_{n}_{z}",
                                    tag="yt")
                    nc.vector.tensor_tensor(
                        tmp[:], ps2[z][:], spill_sb[(n, z)][:],
                        op=mybir.AluOpType.add,
                    )
                    nc.scalar.activation(
                        yt[:], tmp[:],
                        mybir.ActivationFunctionType.Relu, scale=G_EVAC,
                    )
                    st_eng = (nc.scalar, nc.gpsimd, nc.scalar,
                              nc.gpsimd)[z]
                    st_eng.dma_start(
                        out=y_d[rows, n * 512:(n + 1) * 512], in_=yt[:],
                    )

            def emit_p1_half(h):
                # tile 0 runs as two half-width subtiles so most of its work
                # retires before the (xh+mh0)-delivery pin on the startup
                # critical path; spills land in the matching half of the
                # full-width spill tile
                cbase = 128 + h * 256
                ps_half[h] = {}
                for z in range(ZT):
                    ps = pspool.tile([128, 512], F32, name=f"ps0{h}_{z}",
                                     tag="ps")
                    ps_half[h][z] = ps
                    nc.tensor.matmul(
                        ps[:, 0:256], cr_sb[:, :, 0:128],
                        cr_sb[:, :, cbase:cbase + 256],
                        start=True, stop=False,
                        perf_mode=mybir.MatmulPerfMode.DoubleRow,
                    )
                for j in range(JT):
                    msl = mh0h_tiles[(h, j // CH)][:, j % CH, :, :]
                    for z in range(ZT):
                        nc.tensor.matmul(
                            ps_half[h][z][:, 0:256],
                            xh_sb[:, j, :, z * 128:(z + 1) * 128],
                            msl,
                            start=False, stop=(j == JT - 1),
                            perf_mode=mybir.MatmulPerfMode.DoubleRow,
                        )

            def emit_spill_half(h):
                for z in range(ZT):
                    if h == 0:
                        sp = cpool.tile([128, 512], F32, name=f"spill_0_{z}")
                        spill_sb[(0, z)] = sp
                    cols = slice(h * 256, (h + 1) * 256)
                    if z % 2 == 0:
                        nc.scalar.copy(spill_sb[(0, z)][:, cols],
                                       ps_half[h][z][:, 0:256])
                    else:
                        nc.vector.tensor_scalar_add(
                            spill_sb[(0, z)][:, cols],
                            ps_half[h][z][:, 0:256], 0.0)

            def emit_p2_evac_7_half(h):
                # tile 7 as two half-width subtiles: the trailing PE work
                # after the final qq chunk lands is one half-subtile's z
                # sweep instead of a full tile's
                ps2 = {}
                for z in range(ZT):
                    ps2[z] = pspool.tile([128, 512], F32,
                                         name=f"ps27{h}_{z}", tag="ps")
                n = NT - 1
                for z in range(ZT):
                    rows = slice(z * 128, (z + 1) * 128)
                    cols = slice(h * 256, (h + 1) * 256)
                    for j in range(JT):
                        msl = qq7h_tiles[(h, j // CH)][:, j % CH, :, :]
                        nc.tensor.matmul(
                            ps2[z][:, 0:256],
                            bb_sb[:, j, :, z * 128:(z + 1) * 128],
                            msl,
                            start=(j == 0), stop=(j == JT - 1),
                            perf_mode=mybir.MatmulPerfMode.DoubleRow,
                        )
                    tmp = tmppool.tile([128, 512], F32, name=f"tmp7{h}_{z}",
                                       tag="tmp")
                    yt = ypool.tile([128, 512], F16, name=f"yt7{h}_{z}",
                                    tag="yt")
                    nc.vector.tensor_tensor(
                        tmp[:, 0:256], ps2[z][:, 0:256],
                        spill_sb[(n, z)][:, cols],
                        op=mybir.AluOpType.add,
                    )
                    if z % 2 == 0:
                        nc.scalar.activation(
                            yt[:, 0:256], tmp[:, 0:256],
                            mybir.ActivationFunctionType.Relu, scale=G_EVAC,
                        )
                    else:
                        nc.vector.tensor_scalar(
                            yt[:, 0:256], tmp[:, 0:256], G_EVAC, 0.0,
                            mybir.AluOpType.mult, mybir.AluOpType.max,
                        )
                    st_eng = (nc.scalar, nc.sync, nc.scalar, nc.sync)[z]
                    st_eng.dma_start(
                        out=y_d[rows, n * 512 + h * 256:
                                n * 512 + (h + 1) * 256],
                        in_=yt[:, 0:256],
                    )

            # Two-phase schedule with PSUM spilling. Phase 1: all eight Mh
            # passes back-to-back (the spill breaks the 8-bank pipeline-depth
            # wall); DMA streams xh, mh0..mh7 with no x-side stall. Phase 2:
            # all Q passes, consuming bb, qq0..qq7; evacuation folds the
            # spilled partial back in. Both phases are PE-bound vs their DMA
            # streams, so the PE runs stall-free after the initial fill.
            # startup loads: xh chunks interleaved with tile-0 half chunks
            # (8 half-chunks of 256KB, h0's four first)
            for c in range(NCH):
                load_xside(xh_sb, xh_d, c)
                for rep in range(2):
                    k = c * 2 + rep
                    h, cc = k // NCH, k % NCH
                    t = mpool.tile([128, CH, 2, 256], FP8,
                                   name=f"mh0h_{h}_{cc}", tag="mth", bufs=8)
                    nc.sync.dma_start(
                        out=t[:],
                        in_=mh0h_d[:, h, cc * CH:(cc + 1) * CH, :, :],
                    )
                    mh0h_tiles[(h, cc)] = t
            emit_p1_half(0)
            emit_p1_half(1)
            emit_spill_half(0)
            for n in range(1, NT):
                for c in range(NCH):
                    load_mside(mh_tiles, mh_d, n, c, "mh")
                emit_p1(n)
                if n == 1:
                    emit_spill_half(1)
                else:
                    emit_spill(n - 1)
            emit_spill(NT - 1)
            for c in range(NCH):
                load_xside(bb_sb, bb_d, c)
            for n in range(NT - 1):
                for c in range(NCH):
                    load_mside(qq_tiles, qq_d, n, c, "qq")
                emit_p2_evac(n)
            for h in (0, 1):
                for cc in range(NCH):
                    t = mpool.tile([128, CH, 2, 256], FP8,
                                   name=f"qq7h_{h}_{cc}", tag="mth", bufs=8)
                    nc.sync.dma_start(
                        out=t[:],
                        in_=qq7h_d[:, h, cc * CH:(cc + 1) * CH, :, :],
                    )
                    qq7h_tiles[(h, cc)] = t
            emit_p2_evac_7_half(0)
            emit_p2_evac_7_half(1)
    nc.compile()
    return nc


def _materialize_dense(core0, core1, core2, core3) -> np.ndarray:
    """M[(a0,a1,a2,a3),(b0,b1,b2,b3)] from TT cores [r,a,b,q], row-major."""
    t = np.asarray(core0, np.float64).reshape(8, 8, 16)        # a0,b0,r1
    t = np.tensordot(t, np.asarray(core1, np.float64), axes=([2], [0]))
    t = np.tensordot(t, np.asarray(core2, np.float64), axes=([4], [0]))
    t = np.tensordot(t, np.asarray(core3, np.float64), axes=([6], [0]))[..., 0]
    return np.ascontiguousarray(
        t.transpose(0, 2, 4, 6, 1, 3, 5, 7).reshape(D, D)
    )


def _f8(a):
    return np.asarray(a, np.float32).astype(F8)


def _pack_kmajor(a, ncols):
    """[K, ncols] -> [128, K//256, 2, ncols] with k = j*256 + i*128 + p."""
    return np.ascontiguousarray(
        a.reshape(JT, 2, 128, ncols).transpose(2, 0, 1, 3)
    )


_module_cache: list = []


def kernel(x, core0, core1, core2, core3, b):
    M = _materialize_dense(core0, core1, core2, core3)
    x = np.asarray(x, np.float64)
    b64 = np.asarray(b, np.float64)

    s = S_SPLIT
    Mh8 = _f8(SM * M)
    Mh = Mh8.astype(np.float64) / SM
    Q8 = _f8(SQ * (Mh + (M - Mh) / s))

    # [128, NT, JT, 2, 512] fp8 streams
    def pack_mside(m8):
        return np.ascontiguousarray(
            m8.reshape(JT, 2, 128, NT, 512).transpose(2, 3, 0, 1, 4)
        )

    mh_p = pack_mside(Mh8)
    qq_p = pack_mside(Q8)

    # half-width repacks: [128, 2(half), JT, 2, 256]
    def pack_half(mp, n):
        return np.ascontiguousarray(
            mp[:, n].reshape(128, JT, 2, 2, 256).transpose(0, 3, 1, 2, 4)
        )

    mh0h_p = pack_half(mh_p, 0)
    qq7h_p = pack_half(qq_p, NT - 1)

    # bias: y += g * 2 * (o * brow_half), brow duplicated across the two
    # DoubleRow k-slots; pick the fp8 value o minimizing bias error
    o_grid = np.unique(np.abs(
        np.arange(16, 241, dtype=np.float32).astype(F8).astype(np.float32)))
    best = None
    for o in o_grid:
        if o <= 0:
            continue
        beta = _f8(b64 / (2.0 * G_EVAC * o))
        err = np.abs(2.0 * o * G_EVAC * beta.astype(np.float64) - b64).max()
        if best is None or err < best[0]:
            best = (err, float(o), beta)
    _, o_val, beta8 = best
    cr = np.zeros((2, 128 + D), F8)
    cr[:, 0:128] = np.float32(o_val).astype(F8)
    cr[0, 128:] = beta8
    cr[1, 128:] = beta8
    cr = cr.reshape(1, 2, 128 + D)

    in_maps = []
    for c in range(N_CORES):
        xc = x[c * BL:(c + 1) * BL]                   # [BL, D]
        xh8 = _f8(SX * xc)
        xh = xh8.astype(np.float64) / SX
        B8 = _f8(SB * (s * xh + (xc - xh)))
        in_maps.append({
            "xh": _pack_kmajor(xh8.T, BL),
            "bb": _pack_kmajor(B8.T, BL),
            "mh": mh_p,
            "qq": qq_p,
            "cr": cr,
            "mh0h": mh0h_p,
            "qq7h": qq7h_p,
        })

    if not _module_cache:
        _module_cache.append(_build_module())
    nc = _module_cache[0]
    res = run_bass_kernel_spmd(nc, in_maps, core_ids=list(range(N_CORES)))
    out = np.concatenate(
        [res.results[c]["y"].astype(np.float32) for c in range(N_CORES)],
        axis=0,
    )
    return out
